# revision 1
# baseline (speedup 1.0000x reference)
# Laplacian normalization kernel for Trainium2 (8 NeuronCores, SPMD).
#
# out = d^-1/2[:, None] * A * d^-1/2[None, :],  d_i = sum_j A[i, j],  A: [8192, 8192] f32
#
# Sharding: row-wise across 8 cores (1024 rows each). Row sums are local;
# the column-scale vector needs the full d^-1/2 [8192], obtained with a tiny
# AllGather (4KB per core). Each core makes two passes over its shard:
#   pass 1: stream 8 tiles of [128, 8192], DVE row-reduce into d_local
#   (rsqrt via ACT sqrt + DVE reciprocal, AllGather, partition-broadcast)
#   pass 2: out_tile = (A_tile * r_row) * c_col in ONE fused DVE op
#           (scalar_tensor_tensor), DMA out.
# The first NCACHE tiles stay resident in SBUF between the passes so their
# pass-2 reload is free (SBUF = 224KB/partition; each tile is 32KB/partition).

import numpy as np

N = 8192
NCORES = 8
R = N // NCORES  # 1024 rows per core
P = 128          # SBUF partitions
T = R // P       # 8 row-tiles of [128, 8192] per core
NCACHE = 3       # tiles kept resident in SBUF between passes

_cache = {}


def _build():
    import concourse.bacc as bacc
    import concourse.mybir as mybir
    import concourse.tile as tile

    f32 = mybir.dt.float32

    nc = bacc.Bacc(
        "TRN2", target_bir_lowering=False, debug=False, num_devices=NCORES
    )
    a = nc.dram_tensor("a_shard", [R, N], f32, kind="ExternalInput").ap()
    out = nc.dram_tensor("out_shard", [R, N], f32, kind="ExternalOutput").ap()

    a_t = a.rearrange("(t p) n -> t p n", p=P)
    o_t = out.rearrange("(t p) n -> t p n", p=P)

    with tile.TileContext(nc) as tc:
        with (
            tc.tile_pool(name="cpool", bufs=1) as cpool,
            tc.tile_pool(name="spool", bufs=2) as spool,
            tc.tile_pool(name="vpool", bufs=1) as vpool,
            tc.tile_pool(name="dram", bufs=1, space="DRAM") as dram,
        ):
            dsum = vpool.tile([P, T], f32, tag="dsum")
            dsqrt = vpool.tile([P, T], f32, tag="dsqrt")
            dinv = vpool.tile([P, T], f32, tag="dinv")
            cvec = vpool.tile([P, N], f32, tag="cvec")
            dloc = dram.tile([1, R], f32, tag="dloc")
            dfull = dram.tile([1, N], f32, tag="dfull")

            cached = {}
            # pass 1: row sums
            for t in range(T):
                if t < NCACHE:
                    tl = cpool.tile([P, N], f32, tag=f"c{t}")
                    cached[t] = tl
                else:
                    tl = spool.tile([P, N], f32, tag="s")
                nc.sync.dma_start(out=tl[:, :], in_=a_t[t])
                nc.vector.reduce_sum(
                    out=dsum[:, t : t + 1], in_=tl[:, :], axis=mybir.AxisListType.X
                )

            # d^-1/2 (ACT Rsqrt is banned for accuracy; sqrt+reciprocal instead)
            nc.scalar.sqrt(dsqrt[:, :], dsum[:, :])
            nc.vector.reciprocal(dinv[:, :], dsqrt[:, :])

            # dinv[p, t] is row t*128+p -> row-ordered [R] vector in DRAM
            nc.sync.dma_start(
                out=dloc[0, :].rearrange("(t p) -> p t", p=P), in_=dinv[:, :]
            )

            nc.gpsimd.collective_compute(
                "AllGather",
                mybir.AluOpType.bypass,
                replica_groups=[list(range(NCORES))],
                ins=[dloc[0, :].opt()],
                outs=[dfull[0, :].opt()],
            )

            # replicate the gathered vector across all 128 partitions
            nc.gpsimd.dma_start(
                out=cvec[:, :], in_=dfull[:, :].to_broadcast((P, N))
            )

            # pass 2: out = (A * r) * c, fused on DVE, in place
            for t in range(T):
                tl = cached.get(t)
                if tl is None:
                    tl = spool.tile([P, N], f32, tag="s")
                    nc.sync.dma_start(out=tl[:, :], in_=a_t[t])
                nc.vector.scalar_tensor_tensor(
                    out=tl[:, :],
                    in0=tl[:, :],
                    scalar=dinv[:, t : t + 1],
                    in1=cvec[:, :],
                    op0=mybir.AluOpType.mult,
                    op1=mybir.AluOpType.mult,
                )
                nc.sync.dma_start(out=o_t[t], in_=tl[:, :])

    nc.compile()
    return nc


def kernel(adjacency_matrix, _trace=False):
    from concourse.bass_utils import run_bass_kernel_spmd

    A = np.ascontiguousarray(np.asarray(adjacency_matrix, dtype=np.float32))
    assert A.shape == (N, N), A.shape

    if "nc" not in _cache:
        _cache["nc"] = _build()
    nc = _cache["nc"]

    in_maps = [{"a_shard": A[c * R : (c + 1) * R]} for c in range(NCORES)]
    res = run_bass_kernel_spmd(
        nc, in_maps, core_ids=list(range(NCORES)), trace=_trace
    )
    _cache["last"] = res
    return np.concatenate(
        [res.results[c]["out_shard"] for c in range(NCORES)], axis=0
    )


# revision 6
# speedup vs baseline: 28601.4791x; 28601.4791x over previous
# Laplacian normalization kernel for Trainium2 (8 NeuronCores, SPMD).
#
# out = d^-1/2[:, None] * A * d^-1/2[None, :],  d_i = sum_j A[i, j],  A: [8192, 8192] f32
#
# Sharding: row-wise across 8 cores (1024 rows each). Row sums are local;
# the column-scale vector needs the full d^-1/2 [8192], obtained with a tiny
# AllGather (4KB per core). Each core makes two passes over its shard:
#   pass 1: stream 8 row-tiles of [128, 8192], DVE row-reduce; the rsqrt and
#           the scatter of each tile's d^-1/2 chunk into the collective input
#           buffer are pipelined per tile so the AllGather can start almost
#           immediately after the last reduce.
#   pass 2: out_tile = (A_tile * r_row) * c_col in ONE fused DVE op
#           (scalar_tensor_tensor), DMA out.
# The first NCACHE row-tiles stay resident in SBUF between passes (their
# pass-2 reload is free); the rest stream through half-tiles of [128, 4096]
# so prefetch can fill the DMA idle window during the collective.
#
# SBUF/partition: 4*32KB cached + 32KB cvec + 3*16KB stream + small = ~216KB
# of 224KB.

import numpy as np

N = 8192
NCORES = 8
R = N // NCORES  # 1024 rows per core
P = 128          # SBUF partitions
T = R // P       # 8 row-tiles of [128, 8192] per core
NCACHE = 4       # row-tiles kept resident in SBUF between passes
NCHUNK = 4       # column chunks per streamed tile
H = N // NCHUNK  # stream chunk width (2048 columns)

_cache = {}


def _build():
    import concourse.bacc as bacc
    import concourse.mybir as mybir
    import concourse.tile as tile

    f32 = mybir.dt.float32
    X = mybir.AxisListType.X
    mult = mybir.AluOpType.mult

    nc = bacc.Bacc(
        "TRN2", target_bir_lowering=False, debug=False, num_devices=NCORES
    )
    a = nc.dram_tensor("a_shard", [R, N], f32, kind="ExternalInput").ap()
    out = nc.dram_tensor("out_shard", [R, N], f32, kind="ExternalOutput").ap()

    a_t = a.rearrange("(t p) n -> t p n", p=P)
    o_t = out.rearrange("(t p) n -> t p n", p=P)

    with tile.TileContext(nc) as tc:
        with (
            tc.tile_pool(name="cpool", bufs=1) as cpool,
            tc.tile_pool(name="spool", bufs=4) as spool,
            tc.tile_pool(name="vpool", bufs=1) as vpool,
            tc.tile_pool(name="dram", bufs=1, space="DRAM") as dram,
        ):
            dsum = vpool.tile([P, T], f32, tag="dsum")
            dsqrt = vpool.tile([P, T], f32, tag="dsqrt")
            dinv = vpool.tile([P, T], f32, tag="dinv")
            hpart = vpool.tile([P, NCHUNK * T], f32, tag="hpart")
            cvec = vpool.tile([P, N], f32, tag="cvec")
            dloc = dram.tile([1, R], f32, tag="dloc")
            dfull = dram.tile([1, N], f32, tag="dfull")

            def finish_row_chunk(t):
                # d^-1/2 for this tile's 128 rows, scattered into the
                # AllGather input as soon as the row sums are ready
                nc.scalar.sqrt(dsqrt[:, t : t + 1], dsum[:, t : t + 1])
                nc.vector.reciprocal(dinv[:, t : t + 1], dsqrt[:, t : t + 1])
                nc.sync.dma_start(
                    out=dloc[0, t * P : (t + 1) * P], in_=dinv[:, t : t + 1]
                )

            cached = {}
            # pass 1: row sums
            for t in range(T):
                if t < NCACHE:
                    tl = cpool.tile([P, N], f32, tag=f"c{t}")
                    cached[t] = tl
                    nc.sync.dma_start(out=tl[:, :], in_=a_t[t])
                    nc.vector.reduce_sum(
                        out=dsum[:, t : t + 1], in_=tl[:, :], axis=X
                    )
                else:
                    for h in range(NCHUNK):
                        tl = spool.tile([P, H], f32, tag="s")
                        nc.sync.dma_start(
                            out=tl[:, :], in_=a_t[t][:, h * H : (h + 1) * H]
                        )
                        nc.vector.reduce_sum(
                            out=hpart[:, NCHUNK * t + h : NCHUNK * t + h + 1],
                            in_=tl[:, :],
                            axis=X,
                        )
                    nc.vector.reduce_sum(
                        out=dsum[:, t : t + 1],
                        in_=hpart[:, NCHUNK * t : NCHUNK * t + NCHUNK],
                        axis=X,
                    )
                finish_row_chunk(t)

            nc.gpsimd.collective_compute(
                "AllGather",
                mybir.AluOpType.bypass,
                replica_groups=[list(range(NCORES))],
                ins=[dloc[0, :].opt()],
                outs=[dfull[0, :].opt()],
            )

            # replicate the gathered vector across all 128 partitions
            nc.gpsimd.dma_start(
                out=cvec[:, :], in_=dfull[:, :].to_broadcast((P, N))
            )

            # pass 2: out = (A * r) * c fused on DVE; uncached tiles stream
            # in column halves; end on cached tiles so the tail is store-only
            order = []
            un = [t for t in range(T) if t >= NCACHE]
            ca = [t for t in range(T) if t < NCACHE]
            while un or ca:
                if un:
                    order.append(un.pop(0))
                if ca:
                    order.append(ca.pop(0))
            for t in order:
                tl = cached.get(t)
                if tl is not None:
                    nc.vector.scalar_tensor_tensor(
                        out=tl[:, :],
                        in0=tl[:, :],
                        scalar=dinv[:, t : t + 1],
                        in1=cvec[:, :],
                        op0=mult,
                        op1=mult,
                    )
                    nc.sync.dma_start(out=o_t[t], in_=tl[:, :])
                else:
                    for h in range(NCHUNK):
                        tl = spool.tile([P, H], f32, tag="s")
                        nc.sync.dma_start(
                            out=tl[:, :], in_=a_t[t][:, h * H : (h + 1) * H]
                        )
                        nc.vector.scalar_tensor_tensor(
                            out=tl[:, :],
                            in0=tl[:, :],
                            scalar=dinv[:, t : t + 1],
                            in1=cvec[:, h * H : (h + 1) * H],
                            op0=mult,
                            op1=mult,
                        )
                        nc.sync.dma_start(
                            out=o_t[t][:, h * H : (h + 1) * H], in_=tl[:, :]
                        )

    nc.compile()
    return nc


def kernel(adjacency_matrix, _trace=False):
    from concourse.bass_utils import run_bass_kernel_spmd

    A = np.ascontiguousarray(np.asarray(adjacency_matrix, dtype=np.float32))
    assert A.shape == (N, N), A.shape

    if "nc" not in _cache:
        _cache["nc"] = _build()
    nc = _cache["nc"]

    in_maps = [{"a_shard": A[c * R : (c + 1) * R]} for c in range(NCORES)]
    res = run_bass_kernel_spmd(
        nc, in_maps, core_ids=list(range(NCORES)), trace=_trace
    )
    _cache["last"] = res
    return np.concatenate(
        [res.results[c]["out_shard"] for c in range(NCORES)], axis=0
    )


# revision 10
# speedup vs baseline: 35943.1755x; 1.2567x over previous
# Laplacian normalization kernel for Trainium2 (8 NeuronCores, SPMD).
#
# out = d^-1/2[:, None] * A * d^-1/2[None, :],  d_i = sum_j A[i, j],  A: [8192, 8192] f32
#
# Sharding: row-wise across 8 cores (1024 rows each). Row sums are local; the
# column-scale vector needs the full d^-1/2 [8192], obtained with a tiny
# AllGather (4KB per core). Two passes over the shard per core:
#   pass 1: row sums.  All DMA and DVE work is uniform [128, 2048] chunks
#           (1MB loads, 2.2us reduces) so the DVE queue never head-of-line
#           blocks the DMA slot recycling.
#   middle: rsqrt on [128, 8] (ACT sqrt + DVE reciprocal), PE-transpose to
#           [8, 128] so the collective input is written with ONE contiguous
#           4KB DMA (a strided [128,1]-per-tile scatter fragments into 4-byte
#           DMA descriptors), AllGather, then broadcast the gathered vector
#           across partitions in 4 chunked DMAs so pass-2 compute on chunk c
#           only waits for broadcast chunk c.
#   pass 2: out = (A * r_row) * c_col in one fused DVE op per chunk
#           (scalar_tensor_tensor), store per chunk.
# The first NCACHE row-tiles stay resident in SBUF between the passes (their
# pass-2 reload is free); the rest re-stream through 5 rotating chunk slots,
# which also serve as prefetch during the collective window.
#
# SBUF/partition: 4*32KB cached + 5*8KB stream + 32KB cvec + ~1KB small
# = ~201KB of the ~208KB Tile exposes.

import numpy as np

N = 8192
NCORES = 8
R = N // NCORES  # 1024 rows per core
P = 128          # SBUF partitions
T = R // P       # 8 row-tiles of [128, 8192] per core
NCACHE = 4       # row-tiles kept resident in SBUF between passes
NCHUNK = 4       # column chunks per row-tile
H = N // NCHUNK  # chunk width (2048 columns = 1MB per [128, H] chunk)

_cache = {}


def _build():
    import concourse.bacc as bacc
    import concourse.mybir as mybir
    import concourse.tile as tile
    from concourse import masks

    f32 = mybir.dt.float32
    X = mybir.AxisListType.X
    mult = mybir.AluOpType.mult

    nc = bacc.Bacc(
        "TRN2", target_bir_lowering=False, debug=False, num_devices=NCORES
    )
    a = nc.dram_tensor("a_shard", [R, N], f32, kind="ExternalInput").ap()
    out = nc.dram_tensor("out_shard", [R, N], f32, kind="ExternalOutput").ap()

    a_t = a.rearrange("(t p) n -> t p n", p=P)
    o_t = out.rearrange("(t p) n -> t p n", p=P)

    def hs(h):
        return slice(h * H, (h + 1) * H)

    with tile.TileContext(nc) as tc:
        with (
            tc.tile_pool(name="cpool", bufs=1) as cpool,
            tc.tile_pool(name="spool", bufs=5) as spool,
            tc.tile_pool(name="vpool", bufs=1) as vpool,
            tc.tile_pool(name="psum", bufs=1, space="PSUM") as psum,
            tc.tile_pool(name="dram", bufs=1, space="DRAM") as dram,
        ):
            dsum = vpool.tile([P, T], f32, tag="dsum")
            dinv = vpool.tile([P, T], f32, tag="dinv")
            hpart = vpool.tile([P, NCHUNK * T], f32, tag="hpart")
            cvec = vpool.tile([P, N], f32, tag="cvec")
            ident = vpool.tile([P, P], f32, tag="ident")
            dinv_tp = vpool.tile([T, P], f32, tag="dinv_tp")
            dinv_tpp = psum.tile([T, P], f32, tag="dinv_tpp")
            dloc = dram.tile([1, R], f32, tag="dloc")
            dfull = dram.tile([1, N], f32, tag="dfull")

            masks.make_identity(nc, ident[:, :])

            cached = {}
            # pass 1: row sums, uniform [128, H] chunks
            for t in range(T):
                if t < NCACHE:
                    big = cpool.tile([P, N], f32, tag=f"c{t}")
                    cached[t] = big
                for h in range(NCHUNK):
                    if t < NCACHE:
                        tl = cached[t][:, hs(h)]
                    else:
                        stile = spool.tile([P, H], f32, tag="s")
                        tl = stile[:, :]
                    nc.sync.dma_start(out=tl, in_=a_t[t][:, hs(h)])
                    c = NCHUNK * t + h
                    nc.vector.reduce_sum(
                        out=hpart[:, c : c + 1], in_=tl, axis=X
                    )
                nc.vector.reduce_sum(
                    out=dsum[:, t : t + 1],
                    in_=hpart[:, NCHUNK * t : NCHUNK * (t + 1)],
                    axis=X,
                )

            # d^-1/2 (ACT Rsqrt is banned for accuracy; sqrt+reciprocal), then
            # PE-transpose [128, T] -> [T, 128] so the collective input DMA is
            # one contiguous row-ordered write
            nc.scalar.sqrt(dsum[:, :], dsum[:, :])
            nc.vector.reciprocal(dinv[:, :], dsum[:, :])
            nc.tensor.transpose(dinv_tpp[:, :], dinv[:, :], ident[:, :])
            nc.scalar.copy(dinv_tp[:, :], dinv_tpp[:, :])
            nc.gpsimd.dma_start(out=dloc[0, :], in_=dinv_tp[:, :])

            nc.gpsimd.collective_compute(
                "AllGather",
                mybir.AluOpType.bypass,
                replica_groups=[list(range(NCORES))],
                ins=[dloc[0, :].opt()],
                outs=[dfull[0, :].opt()],
            )

            # replicate the gathered vector across all 128 partitions, chunked
            # so pass-2 chunk c only waits for broadcast chunk c
            for h in range(NCHUNK):
                nc.sync.dma_start(
                    out=cvec[:, hs(h)],
                    in_=dfull[0:1, hs(h)].to_broadcast((P, H)),
                )

            # pass 2: out = (A * r) * c fused on DVE per chunk; stream tiles
            # interleaved with cached, ending on cached (tail is store-only)
            order = []
            un = [t for t in range(T) if t >= NCACHE]
            ca = [t for t in range(T) if t < NCACHE]
            while un or ca:
                if un:
                    order.append(un.pop(0))
                if ca:
                    order.append(ca.pop(0))
            for t in order:
                for h in range(NCHUNK):
                    if t in cached:
                        tl = cached[t][:, hs(h)]
                    else:
                        stile = spool.tile([P, H], f32, tag="s")
                        tl = stile[:, :]
                        nc.sync.dma_start(out=tl, in_=a_t[t][:, hs(h)])
                    nc.vector.scalar_tensor_tensor(
                        out=tl,
                        in0=tl,
                        scalar=dinv[:, t : t + 1],
                        in1=cvec[:, hs(h)],
                        op0=mult,
                        op1=mult,
                    )
                    nc.sync.dma_start(out=o_t[t][:, hs(h)], in_=tl)

    nc.compile()
    return nc


def kernel(adjacency_matrix, _trace=False):
    from concourse.bass_utils import run_bass_kernel_spmd

    A = np.ascontiguousarray(np.asarray(adjacency_matrix, dtype=np.float32))
    assert A.shape == (N, N), A.shape

    if "nc" not in _cache:
        _cache["nc"] = _build()
    nc = _cache["nc"]

    in_maps = [{"a_shard": A[c * R : (c + 1) * R]} for c in range(NCORES)]
    res = run_bass_kernel_spmd(
        nc, in_maps, core_ids=list(range(NCORES)), trace=_trace
    )
    _cache["last"] = res
    return np.concatenate(
        [res.results[c]["out_shard"] for c in range(NCORES)], axis=0
    )


# revision 11
# speedup vs baseline: 37539.6720x; 1.0444x over previous
# Laplacian normalization kernel for Trainium2 (8 NeuronCores, SPMD).
#
# out = d^-1/2[:, None] * A * d^-1/2[None, :],  d_i = sum_j A[i, j],  A: [8192, 8192] f32
#
# Sharding: row-wise across 8 cores (1024 rows each). Row sums are local; the
# column-scale vector needs the full d^-1/2 [8192], obtained with a tiny
# AllGather (4KB per core). Two passes over the shard per core:
#   pass 1: row sums in uniform small chunks (so the in-order DVE queue never
#           head-of-line blocks DMA slot recycling).
#   middle: rsqrt on [128, 8] (ACT sqrt + DVE reciprocal), PE-transpose to
#           [8, 128] so the collective input is written with ONE contiguous
#           4KB DMA (a [128,1]-per-tile scatter fragments into 4-byte DMA
#           descriptors), AllGather, then broadcast the gathered vector
#           across partitions in 4 chunked DMAs so pass-2 compute on chunk c
#           only waits for broadcast chunk c.
#   pass 2: out = (A * r_row) * c_col in one fused DVE op per chunk
#           (scalar_tensor_tensor), store per chunk.
#
# Queue discipline: ALL loads go on the Sync HWDGE queue; the broadcast and
# ALL stores go on the Scalar HWDGE queue. HWDGE queues execute in order, so
# putting the (collective-gated) broadcast on the load queue would block
# pass-2 prefetch from filling the otherwise-dead DMA window during the
# collective rendezvous.
#
# The first NCACHE row-tiles stay resident in SBUF between the passes (their
# pass-2 reload is free); the rest re-stream through 5 rotating 1MB chunk
# slots, which double as prefetch buffers during the collective window.
#
# SBUF/partition: 4*32KB cached + 5*8KB stream + 32KB cvec + ~1KB small
# = ~201KB of the ~208KB Tile exposes.

import numpy as np

N = 8192
NCORES = 8
R = N // NCORES  # 1024 rows per core
P = 128          # SBUF partitions
T = R // P       # 8 row-tiles of [128, 8192] per core
NCACHE = 4       # row-tiles kept resident in SBUF between passes
NCHUNK = 4       # column chunks per streamed row-tile (1MB each)
H = N // NCHUNK  # stream chunk width (2048 columns)
CCH = 2          # column chunks per cached row-tile (2MB each)
CH = N // CCH    # cached chunk width (4096 columns)

_cache = {}


def _build():
    import concourse.bacc as bacc
    import concourse.mybir as mybir
    import concourse.tile as tile
    from concourse import masks

    f32 = mybir.dt.float32
    X = mybir.AxisListType.X
    mult = mybir.AluOpType.mult

    nc = bacc.Bacc(
        "TRN2", target_bir_lowering=False, debug=False, num_devices=NCORES
    )
    a = nc.dram_tensor("a_shard", [R, N], f32, kind="ExternalInput").ap()
    out = nc.dram_tensor("out_shard", [R, N], f32, kind="ExternalOutput").ap()

    a_t = a.rearrange("(t p) n -> t p n", p=P)
    o_t = out.rearrange("(t p) n -> t p n", p=P)

    with tile.TileContext(nc) as tc:
        with (
            tc.tile_pool(name="cpool", bufs=1) as cpool,
            tc.tile_pool(name="spool", bufs=5) as spool,
            tc.tile_pool(name="vpool", bufs=1) as vpool,
            tc.tile_pool(name="psum", bufs=1, space="PSUM") as psum,
            tc.tile_pool(name="dram", bufs=1, space="DRAM") as dram,
        ):
            dsum = vpool.tile([P, T], f32, tag="dsum")
            dinv = vpool.tile([P, T], f32, tag="dinv")
            hpart = vpool.tile([P, NCHUNK * T], f32, tag="hpart")
            cvec = vpool.tile([P, N], f32, tag="cvec")
            ident = vpool.tile([P, P], f32, tag="ident")
            dinv_tp = vpool.tile([T, P], f32, tag="dinv_tp")
            dinv_tpp = psum.tile([T, P], f32, tag="dinv_tpp")
            dloc = dram.tile([1, R], f32, tag="dloc")
            dfull = dram.tile([1, N], f32, tag="dfull")

            masks.make_identity(nc, ident[:, :])

            cached = {}
            # pass 1: row sums; cached tiles in 2MB chunks, streamed in 1MB
            for t in range(T):
                nch = NCHUNK
                if t < NCACHE:
                    big = cpool.tile([P, N], f32, tag=f"c{t}")
                    cached[t] = big
                    nch = CCH
                w = N // nch
                for h in range(nch):
                    cols = slice(h * w, (h + 1) * w)
                    if t < NCACHE:
                        tl = cached[t][:, cols]
                    else:
                        stile = spool.tile([P, H], f32, tag="s")
                        tl = stile[:, :]
                    nc.sync.dma_start(out=tl, in_=a_t[t][:, cols])
                    c = NCHUNK * t + h
                    nc.vector.reduce_sum(
                        out=hpart[:, c : c + 1], in_=tl, axis=X
                    )
                nc.vector.reduce_sum(
                    out=dsum[:, t : t + 1],
                    in_=hpart[:, NCHUNK * t : NCHUNK * t + nch],
                    axis=X,
                )

            # d^-1/2 (ACT Rsqrt is banned for accuracy; sqrt+reciprocal), then
            # PE-transpose [128, T] -> [T, 128] so the collective input DMA is
            # one contiguous row-ordered 4KB write
            nc.scalar.sqrt(dsum[:, :], dsum[:, :])
            nc.vector.reciprocal(dinv[:, :], dsum[:, :])
            nc.tensor.transpose(dinv_tpp[:, :], dinv[:, :], ident[:, :])
            nc.scalar.copy(dinv_tp[:, :], dinv_tpp[:, :])
            nc.gpsimd.dma_start(out=dloc[0, :], in_=dinv_tp[:, :])

            nc.gpsimd.collective_compute(
                "AllGather",
                mybir.AluOpType.bypass,
                replica_groups=[list(range(NCORES))],
                ins=[dloc[0, :].opt()],
                outs=[dfull[0, :].opt()],
            )

            # replicate the gathered vector across all 128 partitions, chunked
            # so pass-2 chunk c only waits for broadcast chunk c (on the store
            # queue: must NOT block pass-2 prefetch loads on the sync queue)
            for h in range(NCHUNK):
                cols = slice(h * H, (h + 1) * H)
                nc.scalar.dma_start(
                    out=cvec[:, cols],
                    in_=dfull[0:1, cols].to_broadcast((P, H)),
                )

            # pass 2: out = (A * r) * c fused on DVE per chunk; streamed tiles
            # interleaved with cached, ending on cached (tail is store-only)
            order = []
            un = [t for t in range(T) if t >= NCACHE]
            ca = [t for t in range(T) if t < NCACHE]
            while un or ca:
                if un:
                    order.append(un.pop(0))
                if ca:
                    order.append(ca.pop(0))
            for t in order:
                nch = CCH if t in cached else NCHUNK
                w = N // nch
                for h in range(nch):
                    cols = slice(h * w, (h + 1) * w)
                    if t in cached:
                        tl = cached[t][:, cols]
                    else:
                        stile = spool.tile([P, H], f32, tag="s")
                        tl = stile[:, :]
                        nc.sync.dma_start(out=tl, in_=a_t[t][:, cols])
                    nc.vector.scalar_tensor_tensor(
                        out=tl,
                        in0=tl,
                        scalar=dinv[:, t : t + 1],
                        in1=cvec[:, cols],
                        op0=mult,
                        op1=mult,
                    )
                    nc.scalar.dma_start(out=o_t[t][:, cols], in_=tl)

    nc.compile()
    return nc


def kernel(adjacency_matrix, _trace=False):
    from concourse.bass_utils import run_bass_kernel_spmd

    A = np.ascontiguousarray(np.asarray(adjacency_matrix, dtype=np.float32))
    assert A.shape == (N, N), A.shape

    if "nc" not in _cache:
        _cache["nc"] = _build()
    nc = _cache["nc"]

    in_maps = [{"a_shard": A[c * R : (c + 1) * R]} for c in range(NCORES)]
    res = run_bass_kernel_spmd(
        nc, in_maps, core_ids=list(range(NCORES)), trace=_trace
    )
    _cache["last"] = res
    return np.concatenate(
        [res.results[c]["out_shard"] for c in range(NCORES)], axis=0
    )
